# revision 33
# baseline (speedup 1.0000x reference)
"""Masked-softmax cross-entropy loss on 8 Trainium2 cores - PE-bucket design.

Math per target row t (16384 rows, each over 4096 src cols):
  numer[t] = sum_j exp(x[t,j]/tau) over valid src cols j with color == tgt color t
  denom[t] = sum_j exp(x[t,j]/tau) over valid src cols j
  p_gt = numer/denom, nll = -log(p_gt + eps); rows with numer==0 masked out.
Segment aggregation (32 segments) happens on host - it touches 16K scalars.

Device strategy:
  - Host maps colors to small integer ids (exact byte equality), builds a
    per-batch one-hot matrix OH[j, c] over the <=98 unique colors plus a
    "valid" ones-column, quantizes x to int8 (x ~= q * 3/64, quarters HBM
    traffic vs f32), and ships it TRANSPOSED (j on partitions).
  - Device computes et ~= exp(10*x) as bf16, split between two engines:
      ACT tiles:  activation(Exp, scale=10*S) straight from int8
      DVE tiles:  exp bit-hack - tensor_scalar(q*a + b) rounded into int16 and
                  reinterpreted as bf16 gives 2^((i-16256)/128) ~= exp(10*x)
                  to ~3%; the error cancels between numer and denom (validated
                  rel err ~1e-4 on the final loss, tolerance 2e-2).
  - The TensorEngine computes bucket[c, t] = OH^T @ et with PSUM accumulation
    over the 32 j-chunks of 128 (lhsT padded to 128 cols so FWL engages; the
    MM stream runs at its ~216ns/512-col floor).  DVE copies PSUM out (bf16)
    at the end.
  - Host gathers numer[t] = bucket[tgt_color[t], t], denom = bucket[98, t],
    rebuilds the match mask exactly from the ids, and finishes the tiny
    NLL/segment reduction in f32.
Invalid src cols have an all-zero OH row, so they drop out of both numer and
denom exactly; the row mask/count comes from exact host-side id matching, so
quantization can never flip it.

Sharding: core c takes half a batch: batch c//2, tgt-row half c%2 (2048 rows).

Sync-wait budget: walrus allows at most one sem wait per instruction.  Every
x unit gets a private SBUF buffer and its own HWDGE lane so loads carry no
waits; tiny same-engine "interposer" copies absorb the cross-engine waits of
the exp ops (PE et-slot reuse, other-engine WAW, the x-DMA wait), a
standalone ldweights absorbs the OH-load wait for PE, and the kernel-tail
drain is split into one drain per proc.
"""

import os
import numpy as np

B = 4
S_TGT = 8
L_TGT = 512
C = 4
N = 4096          # src columns (= 8*512) = total tgt rows per batch
P = 128
TROWS = 2048      # tgt rows per core (half a batch)
NTILE = 16        # x units per core; unit = 2 j-chunks of 128, [128, 4096] int8
TILEW = 4096      # free width of one x unit (2 chunks x 2048 t)
NCOL = 99         # bucket columns: ids 0..97 + "valid" ones-column at 98
NCOLP = 128       # lhsT padded to 128 cols so FWL (fast weight load) engages
NBUF = 13         # et buffer depth
NCORES = 8
PAD = -1.0
EPS = 1e-15

QS = 3.0 / 64.0             # int8 quantization scale: x ~= q * QS
BH_A = 60.0 / np.log(2.0)   # bit-hack: i16 = q*BH_A + BH_B ; bf16(i16) ~= exp(10*q*QS)
BH_B = 16256.0
ACT_TILES = (2, 4, 6, 8, 10, 12)    # units whose exp runs on ACT; rest on DVE

_NC_CACHE = {}


def _patch_split_drain():
    """Split the kernel-tail drain's sem waits across several drain
    instructions (walrus rejects >1 sync wait on one CTRL instruction)."""
    import concourse.tile as tile
    from concourse.vector_clock import ScopedClock, VectorClock

    if getattr(tile.TileContext, "_split_drain_patched", False):
        return

    def _drain_and_barrier(self, tick_clock, wait_clock):
        g = tick_clock.global_clock
        n = len(g)
        for base in range(n):
            vec = [g[i] if i == base else 0 for i in range(n)]
            if not any(vec):
                continue
            d = self.nc.sync.drain()
            wait_clock.add_sem_waits(d.ins, ScopedClock({None: VectorClock(vec)}))
        self.nc.all_engine_barrier()
        popped = self.nc._tile_sem_poison_stack.pop()
        assert popped is self._sem_poison
        self.nc.clear_and_free_semaphores(list(self.sems.allocated().values()))
        self.nc.all_engine_barrier()

    tile.TileContext._drain_and_barrier = _drain_and_barrier
    tile.TileContext._split_drain_patched = True


def _build_nc():
    import concourse.bass as bass
    import concourse.mybir as mybir
    import concourse.tile as tile
    from concourse.tile_rust import add_dep_helper
    from contextlib import ExitStack

    _patch_split_drain()
    nc = bass.Bass()
    f32 = mybir.dt.float32
    bf16 = mybir.dt.bfloat16
    i8 = mybir.dt.int8
    i16 = mybir.dt.int16

    x = nc.declare_dram_parameter("x", [NTILE * P, TILEW], i8, isOutput=False)
    oh = nc.declare_dram_parameter("oh", [P, 32 * NCOLP], bf16, isOutput=False)
    bucket = nc.declare_dram_parameter("bucket", [NCOLP, TROWS], bf16,
                                       isOutput=True)

    with tile.TileContext(nc) as tc:
        with ExitStack() as ctx:
            const_pool = ctx.enter_context(tc.tile_pool(name="const", bufs=1))
            x_pool = ctx.enter_context(tc.tile_pool(name="x", bufs=NTILE))
            e_pool = ctx.enter_context(tc.tile_pool(name="exps", bufs=NBUF))
            res_pool = ctx.enter_context(tc.tile_pool(name="res", bufs=1))
            psum_pool = ctx.enter_context(
                tc.tile_pool(name="ps", bufs=1, space="PSUM"))

            oh_sb1 = const_pool.tile([P, 32 * NCOLP], bf16)
            nc.gpsimd.dma_start(oh_sb1[:], oh[:])

            def oh_lw(k):
                return oh_sb1[:, k * NCOLP:(k + 1) * NCOLP]
            res_sb = res_pool.tile([NCOLP, TROWS], bf16)
            ps = [psum_pool.tile([NCOLP, 512], f32, name=f"ps{s}", tag=f"ps{s}")
                  for s in range(4)]

            # warm-up absorbers for the oh-DMA wait per engine
            warm = res_pool.tile([P, 4], bf16)
            nc.vector.tensor_copy(warm[:, 0:1], oh_sb1[:, 0:1])
            nc.scalar.copy(warm[:, 1:2], oh_sb1[:, 0:1])
            nc.tensor.ldweights(oh_sb1[:, 0:NCOLP])

            def ecopy(m, purpose, src):
                dst = res_pool.tile([P, 1], bf16, name=f"s{purpose}{m}",
                                    tag=f"s{purpose}{m}")[:]
                if m in ACT_TILES:
                    return nc.scalar.copy(dst, src)
                return nc.vector.tensor_copy(dst, src)

            loads = []
            ewrites = []
            last_mm = []
            for m in range(NTILE):
                # every x unit has its own buffer and HWDGE lane: the load
                # carries no sync waits and streams at full rate
                xt = x_pool.tile([P, TILEW], i8)
                ld = nc.sync.dma_start(xt[:], x[m * P:(m + 1) * P, :])
                loads.append(ld)

                # exp-engine-side absorbers: et-slot WAW vs PE matmuls of
                # m-NBUF, et-slot WAW vs the other engine's write of m-NBUF,
                # then the x-DMA wait (the exp itself keeps at most the
                # bias-const wait on its first ACT instance)
                et = e_pool.tile([P, TILEW], bf16)
                exp_pre = []
                if m >= NBUF:
                    sA = ecopy(m, "a", oh_sb1[:, 0:1])
                    add_dep_helper(sA.ins, last_mm[m - NBUF].ins, sync=True,
                                   reason="absorb et-slot WAW (PE read)")
                    exp_pre.append(sA)
                    if (m in ACT_TILES) != (m - NBUF in ACT_TILES):
                        sB = ecopy(m, "b", oh_sb1[:, 0:1])
                        add_dep_helper(sB.ins, ewrites[m - NBUF].ins, sync=True,
                                       reason="absorb et-slot WAW (other writer)")
                        exp_pre.append(sB)
                exp_pre.append(ecopy(m, "c", xt[:, 0:1]))
                if m in ACT_TILES:
                    ew = nc.scalar.activation(
                        et[:], xt[:], mybir.ActivationFunctionType.Exp,
                        scale=10.0 * QS)
                else:
                    ew = nc.vector.tensor_scalar(
                        out=et[:].bitcast(i16), in0=xt[:],
                        scalar1=float(BH_A), scalar2=float(BH_B),
                        op0=mybir.AluOpType.mult, op1=mybir.AluOpType.add)
                for a in exp_pre:
                    add_dep_helper(ew.ins, a.ins, sync=False,
                                   reason="exp ordered after wait absorber")
                ewrites.append(ew)

                mm = None
                for q in range(2):
                    lw = oh_lw(2 * m + q)
                    for s in range(4):
                        mm = nc.tensor.matmul(
                            ps[s][:],
                            lw,
                            et[:, q * TROWS + s * 512:q * TROWS + (s + 1) * 512],
                            start=(m == 0 and q == 0),
                            stop=(m == NTILE - 1 and q == 1),
                        )
                last_mm.append(mm)

            # tail: casts split across ACT and DVE (both idle here), one
            # output DMA per engine's half so each carries a single wait
            nc.scalar.copy(res_sb[:, 0:512], ps[0][:])
            nc.vector.tensor_copy(res_sb[:, 1024:1536], ps[2][:])
            nc.scalar.copy(res_sb[:, 512:1024], ps[1][:])
            nc.vector.tensor_copy(res_sb[:, 1536:2048], ps[3][:])
            nc.gpsimd.dma_start(bucket[:, 0:1024], res_sb[:, 0:1024])
            nc.gpsimd.dma_start(bucket[:, 1024:2048], res_sb[:, 1024:2048])
    return nc


def _get_nc():
    key = (NBUF, NCOLP, ACT_TILES)
    if key not in _NC_CACHE:
        _NC_CACHE[key] = _build_nc()
    return _NC_CACHE[key]


def _color_ids(src, tgt):
    """Map each color row to a per-batch integer id via exact byte equality."""
    src_f = np.ascontiguousarray(src.reshape(B, -1, C))
    tgt_f = np.ascontiguousarray(tgt.reshape(B, -1, C))
    n_s = src_f.shape[1]
    src_ids = np.empty((B, n_s), np.int32)
    tgt_ids = np.empty((B, tgt_f.shape[1]), np.int32)
    for b in range(B):
        allc = np.ascontiguousarray(np.concatenate([src_f[b], tgt_f[b]], axis=0))
        view = allc.view([("", allc.dtype)] * C).reshape(-1)
        uniq, inv = np.unique(view, return_inverse=True)
        assert len(uniq) <= NCOL - 1, f"too many unique colors: {len(uniq)}"
        ids = inv.astype(np.int32)
        s_ids, t_ids = ids[:n_s].copy(), ids[n_s:].copy()
        s_ids[np.all(src_f[b] == PAD, axis=-1)] = -1
        t_ids[np.all(tgt_f[b] == PAD, axis=-1)] = -2
        src_ids[b], tgt_ids[b] = s_ids, t_ids
    return src_ids, tgt_ids


def kernel(seg_sim_map, seg_colors_src, seg_colors_tgt):
    import ml_dtypes
    from concourse.bass_utils import run_bass_kernel_spmd

    bf16 = ml_dtypes.bfloat16
    seg_sim_map = np.asarray(seg_sim_map, dtype=np.float32)
    src_ids, tgt_ids = _color_ids(
        np.asarray(seg_colors_src, np.float32), np.asarray(seg_colors_tgt, np.float32)
    )

    # per-batch one-hot color matrix [4096 j, NCOL], swizzled for the device
    # as [128, 32*NCOL] so each partition line is contiguous
    oh_maps = []
    for b in range(B):
        ohb = np.zeros((N, NCOLP), np.float32)
        valid = src_ids[b] >= 0
        ohb[np.arange(N)[valid], src_ids[b][valid]] = 1.0
        ohb[valid, NCOL - 1] = 1.0
        oh_maps.append(np.ascontiguousarray(
            ohb.reshape(32, P, NCOLP).transpose(1, 0, 2).reshape(P, 32 * NCOLP)
        ).astype(bf16))

    in_maps = []
    for c in range(NCORES):
        b, h = c // 2, c % 2
        block = seg_sim_map[b, h * TROWS:(h + 1) * TROWS, :]   # [2048 t, 4096 j]
        q = np.clip(np.rint(block.T * np.float32(1.0 / QS)), -127, 127)
        # x^T unit layout: row m*128+p, col q*2048+t = x[(2m+q)*128+p -> j, t]
        xdev = np.ascontiguousarray(
            q.reshape(NTILE, 2, P, TROWS).transpose(0, 2, 1, 3)
            .reshape(NTILE * P, TILEW).astype(np.int8)
        )
        in_maps.append({"x": xdev, "oh": oh_maps[b]})

    trace = os.environ.get("KERNEL_PROFILE", "") == "1"
    nc = _get_nc()
    out = run_bass_kernel_spmd(nc, in_maps, list(range(NCORES)), trace=trace)
    if trace and out.exec_time_ns is not None:
        print(f"HW exec time: {out.exec_time_ns} ns")
        print(f"HW exec mean: {out.mean_exec_time_ns} ns")

    numer = np.empty((B, N), np.float32)
    denom = np.empty((B, N), np.float32)
    matched = np.empty((B, N), np.float32)
    for c in range(NCORES):
        b, h = c // 2, c % 2
        bk = out.results[c]["bucket"].astype(np.float32)       # [NCOLP, 2048]
        tid = tgt_ids[b, h * TROWS:(h + 1) * TROWS]
        safe = np.where(tid >= 0, tid, 0)
        g = bk[safe, np.arange(TROWS)]
        numer[b, h * TROWS:(h + 1) * TROWS] = np.where(tid >= 0, g, 0.0)
        denom[b, h * TROWS:(h + 1) * TROWS] = bk[NCOL - 1]
        present = np.zeros(NCOL, bool)
        present[src_ids[b][src_ids[b] >= 0]] = True
        matched[b, h * TROWS:(h + 1) * TROWS] = np.where(
            tid >= 0, present[safe], False).astype(np.float32)

    # host finalize, mirroring the reference ops in f32 (touches 16K scalars)
    p_gt = numer / denom
    nll = -np.log(p_gt + np.float32(EPS))
    nll3 = nll.reshape(B, S_TGT, L_TGT)
    m3 = matched.reshape(B, S_TGT, L_TGT)
    nvalid = m3.sum(-1)
    seg_loss = np.where(
        nvalid > 0, (nll3 * m3).sum(-1) / np.maximum(nvalid, np.float32(1.0)), 0.0
    ).astype(np.float32)
    cnt = int((nvalid > 0).sum())
    total = np.float32(seg_loss.sum(dtype=np.float32) / np.float32(max(cnt, 1)))
    return np.asarray(total, np.float32), np.asarray(cnt, np.int32)
